# revision 47
# baseline (speedup 1.0000x reference)
"""GATv2 attention-score kernel for 8 Trainium2 NeuronCores.

Reference computation (per b, h):
    scores[i, j] = sum_d silu(q[i, d] + k[j, d]) * a[h, d]
    attn = softmax(where(mask, -inf, scores), axis=-1), zeroed at mask.

Algorithm: the 2-variable map silu(u + v) is approximated by a rank-R
separable expansion  silu(u+v) ~= sum_p f_p(u) * g_p(v)  (weighted SVD
of silu on a Gaussian-weighted grid).  Then

    scores[i, j] ~= sum_{d,p} f_p(q[i,d]) * (g_p(k[j,d]) * a[h,d])

which is a single matmul with contraction K = D*R — the entire ScalarE
silu stage of the direct algorithm (the 109 us/core roofline) is
replaced by a few us of PE time.  R = 8 with features 0-3 in fp16 and
features 4-7 in fp8e4m3 gives max attn rel err ~5e-3 (gate is 2e-2).

Sharding: the 32 (b, h) pairs are split 4-per-core (all four share one
b, so the mask is per-core constant).

Per-core dataflow (B=4, H=8, LQ=LK=256, D=64, R=8 -> 4 K-chunks of
128 = 64 d x 2 features; chunks 0-1 fp16, chunks 2-3 fp8):
  - Host prep: fp16 feature chunks of q (f16) and of k with a_h folded
    in (g16), packed for all 4 (b,h) into one uint8 DRAM tensor (one
    8 KB/partition DMA per iteration on the SP HWDGE queue, bitcast
    views feed the matmuls); fp8 chunks as (128, 2, 256) tensors on
    the Act HWDGE queue; mask * -60000 (128, 512) fp16; id128.
  - Per (b,h) l and 128-query i-tile: 5 accumulating PE matmuls into a
    (128, 256) PSUM tile: id128.T @ mk (adds -60000 at masked entries;
    exp underflows to exactly 0), 2 fp16 + 2 fp8 feature chunks.
  - ScalarE Exp straight from PSUM into an fp16 attn strip, with
    accum_out producing the per-row softmax denominator for free.
  - DVE reciprocal + per-segment tensor_scalar_mul (fp16 2x mode).
  - 8 per-segment out-DMAs on the Pool SWDGE queue.

Softmax skips the max-subtraction: scores are bounded (|s| < 3 for
these inputs, exp(s) < 30 fits fp16) and masked entries are exactly 0.
Fully-masked rows would yield NaN but do not occur (P ~ 2^-256).

Measured: 4.8-7.3 us/core/iter on HW in quiet windows (paired-slope
medians drift up to ~11 us when the shared axon host is loaded).
Baseline direct-silu kernel: 120 us; ScalarE silu roofline of the
direct algorithm: 109 us.  Rel err 4.3e-3 (gate 2e-2); set C16 = 4
(all-fp16, R=8) for ~2.2e-3 or R = 10 all-fp16 for ~9.4e-4.
"""

import numpy as np

B, H, L, D = 4, 8, 256, 64
NCORES = 8
BH = 4          # (b, h) pairs per core
R = 8           # separable rank of the silu(u+v) expansion
C16 = 2         # leading K-chunks (of 128 = 64 d x 2 features) in fp16
C8 = R // 2 - C16           # trailing K-chunks in fp8e4m3
import os as _os
# Tuning knobs (defaults = fastest measured config; env-overridable for A/B.
# Measured on HW per-iter: defaults 4.8-7.3 us; DR=1 12-14 us (DoubleRow is
# slow on real HW despite the cost model); EXPPAIR=1 9.6 us; F8PACK=1 15 us;
# PSB=8/IOB=4 worse than 4/3 in same-process A/B; DRSW=1 gives WRONG results
# (guessed SwInterleave layout) and is not faster -- do not enable.)
DR = _os.environ.get("DR", "0") == "1"   # fp8 chunks via one DoubleRow matmul
OUTQ = _os.environ.get("OUTQ", "pool8")  # out-DMA queue: sp|act|pool|pool8
EXPPAIR = _os.environ.get("EXPPAIR", "0") == "1"   # 4x[128,512] exps + DVE reduce
F8PACK = _os.environ.get("F8PACK", "0") == "1"     # fp8 inside the packed DMA
PSB = int(_os.environ.get("PSB", "4"))             # PSUM pool buffers
IOB = int(_os.environ.get("IOB", "3"))             # fp8 io pool buffers
DRSW = _os.environ.get("DRSW", "0") == "1"  # fp8 DoubleRowSwInterleave matmul
# MASKDVE would drop the mask matmul (PE 5->4 per i-tile) by masking on
# DVE via tensor_tensor_reduce, but that ISA op crashes the exec unit on
# this runtime (NRT_EXEC_UNIT_UNRECOVERABLE, deterministic) — keep off.
MASKDVE = _os.environ.get("MASKDVE", "0") == "1"   # mask on DVE, not PE
# HOSTNORM moves the softmax 1/sum to the host unpack (DVE fully idle,
# out-DMA fires right after each exp). Measured: no speedup — the
# recip/mul tail already overlaps fully in steady state. Kept off.
HOSTNORM = _os.environ.get("HOSTNORM", "0") == "1"  # softmax 1/sum on host
F8Q = _os.environ.get("F8Q", "act")     # f8/g8 DMA trigger engine: act|sp|pool
GRID_N = 801    # feature-table grid
GRID_XM = 7.0   # grid covers [-XM, XM]; inputs are randn, |x| < 5.5
MASK_NEG = np.float32(-60000.0)   # fp16-exact; exp(s - 60000) == 0
# packed bytes per (b, h): f16 (1024) | g16 (1024) [| f8 (512) | g8 (512)]
PLB = 3072 if F8PACK else 2048

_cache = {}
PREC = "fp16"


def _feature_tables():
    """Rank-R separable basis of silu(u+v): weighted SVD on a grid.

    Returns (f_table (N, R), g_table (N, R)) float32.
    """
    if "feat" in _cache:
        return _cache["feat"]
    g = np.linspace(-GRID_XM, GRID_XM, GRID_N)
    dx = g[1] - g[0]
    w = np.exp(-g * g / 2) / np.sqrt(2 * np.pi) + 1e-5
    sw = np.sqrt(w * dx)
    s = g[:, None] + g[None, :]
    M = (s / (1.0 + np.exp(-s))) * sw[:, None] * sw[None, :]
    U, S, Vt = np.linalg.svd(M)
    f = (U[:, :R] * np.sqrt(S[:R])) / sw[:, None]
    gg = (Vt[:R].T * np.sqrt(S[:R])) / sw[:, None]
    _cache["feat"] = (f.astype(np.float32), gg.astype(np.float32))
    return _cache["feat"]


def _interp_features(x, table):
    """Linear interp of the (GRID_N, R) table at x; returns (*x.shape, R)."""
    dx = 2 * GRID_XM / (GRID_N - 1)
    t = np.clip((x + GRID_XM) / dx, 0.0, GRID_N - 1 - 1e-6)
    i0 = t.astype(np.int32)
    frac = (t - i0)[..., None].astype(np.float32)
    return table[i0] * (1.0 - frac) + table[i0 + 1] * frac


def _build_program(reps=1, stages="full"):
    import concourse.mybir as mybir
    from concourse import bacc
    from concourse.tile import TileContext

    DT = mybir.dt.float32
    HT = mybir.dt.float16
    QT = mybir.dt.float8e4
    nc = bacc.Bacc("TRN2", target_bir_lowering=False, debug=False,
                   num_devices=NCORES)

    pk_d = nc.dram_tensor("pk", [128, BH * PLB], mybir.dt.uint8,
                          kind="ExternalInput")
    if not F8PACK:
        f8_shape = [BH, 128, L, C8] if DRSW else [BH, 128, C8, L]
        f8_d = nc.dram_tensor("f8", f8_shape, QT, kind="ExternalInput")
        g8_d = nc.dram_tensor("g8", f8_shape, QT, kind="ExternalInput")
    mk_d = nc.dram_tensor("mk", [128, 2 * L], HT, kind="ExternalInput")
    id_d = nc.dram_tensor("id128", [128, 128], HT, kind="ExternalInput")
    out_d = nc.dram_tensor("out", [128, BH * 2 * L], HT,
                           kind="ExternalOutput")

    with TileContext(nc) as tc:
        with (
            tc.tile_pool(name="io", bufs=2) as io_pool,
            tc.tile_pool(name="io8", bufs=IOB) as io8_pool,
            tc.tile_pool(name="const", bufs=1) as c_pool,
            tc.tile_pool(name="sm", bufs=2) as sm_pool,
            tc.tile_pool(name="psum", bufs=PSB, space="PSUM") as ps_pool,
        ):
            id_t = c_pool.tile([128, 128], HT, tag="id")
            nc.sync.dma_start(id_t[:], id_d[:])
            mk_t = c_pool.tile([128, 2 * L], HT, tag="mk")
            nc.sync.dma_start(mk_t[:], mk_d[:])

            for _rep in range(reps):
                pk_t = io_pool.tile([128, BH * PLB], mybir.dt.uint8,
                                    tag="pk")
                nc.sync.dma_start(pk_t[:], pk_d[:])
                attn = sm_pool.tile([128, BH * 2 * L], HT, tag="attn")
                if not HOSTNORM:
                    recip = sm_pool.tile([128, BH * 2], DT, tag="recip")
                    sums = sm_pool.tile([128, BH * 2], DT, tag="sums")
                if MASKDVE:
                    attnraw = sm_pool.tile([128, BH * 2 * L], HT, tag="araw")
                for l in range(BH):
                    base = l * PLB
                    if F8PACK:
                        fo, go = base + 2048, base + 2560
                    else:
                        f8_sh = [128, L, C8] if DRSW else [128, C8, L]
                        f8_eng = {"act": nc.scalar, "sp": nc.sync,
                                  "pool": nc.gpsimd}[F8Q]
                        f8_t = io8_pool.tile(f8_sh, QT, tag="f8")
                        f8_eng.dma_start(f8_t[:], f8_d[l])
                        g8_t = io8_pool.tile(f8_sh, QT, tag="g8")
                        f8_eng.dma_start(g8_t[:], g8_d[l])
                    if stages == "dma":
                        continue
                    if EXPPAIR:
                        ps2 = ps_pool.tile([128, 2 * L], DT, tag="ps2")
                    for it in range(2):
                        pv = (ps2[:, it * L:(it + 1) * L] if EXPPAIR
                              else ps_pool.tile([128, L], DT, tag="ps",
                                                name="ps")[:])
                        if not MASKDVE:
                            nc.tensor.matmul(
                                pv, lhsT=id_t[:],
                                rhs=mk_t[:, it * L:(it + 1) * L],
                                start=True, stop=False)
                        for c in range(C16):
                            off = base + (c * L + it * 128) * 2
                            nc.tensor.matmul(
                                pv,
                                lhsT=pk_t[:, off:off + 256].bitcast(HT),
                                rhs=pk_t[:, base + 1024 + c * 512:
                                         base + 1024 + (c + 1) * 512
                                         ].bitcast(HT),
                                start=(MASKDVE and c == 0), stop=False)
                        if F8PACK:
                            lhs8 = (pk_t[:, fo + it * 256:fo + it * 256 + 256]
                                    .bitcast(QT)
                                    .rearrange("p (c m) -> p c m", c=2))
                            rhs8 = (pk_t[:, go:go + 512].bitcast(QT)
                                    .rearrange("p (c n) -> p c n", c=2))
                        elif DRSW:
                            lhs8 = (f8_t[:, it * 128:it * 128 + 128, :]
                                    .rearrange("p m c -> p c m"))
                            rhs8 = g8_t[:].rearrange("p n c -> p c n")
                        else:
                            lhs8 = f8_t[:, :, it * 128:it * 128 + 128]
                            rhs8 = g8_t[:]
                        if DRSW:
                            nc.tensor.matmul(
                                pv, lhsT=lhs8, rhs=rhs8,
                                perf_mode=mybir.MatmulPerfMode
                                .DoubleRowSwInterleave,
                                start=False, stop=True)
                        elif DR:
                            nc.tensor.matmul(
                                pv, lhsT=lhs8, rhs=rhs8,
                                perf_mode=mybir.MatmulPerfMode.DoubleRow,
                                start=False, stop=True)
                        elif F8PACK:
                            for c in range(C8):
                                nc.tensor.matmul(
                                    pv,
                                    lhsT=pk_t[:, fo + it * 256 + c * 128:
                                              fo + it * 256 + (c + 1) * 128
                                              ].bitcast(QT),
                                    rhs=pk_t[:, go + c * 256:
                                             go + (c + 1) * 256].bitcast(QT),
                                    start=False, stop=(c == C8 - 1))
                        else:
                            for c in range(C8):
                                nc.tensor.matmul(
                                    pv,
                                    lhsT=f8_t[:, c, it * 128:it * 128 + 128],
                                    rhs=g8_t[:, c, :],
                                    start=False, stop=(c == C8 - 1))
                        seg = l * 2 + it
                        if stages == "mm" or EXPPAIR:
                            continue
                        if MASKDVE:
                            nc.scalar.activation(
                                attnraw[:, seg * L:(seg + 1) * L], pv,
                                mybir.ActivationFunctionType.Exp)
                            nc.vector.tensor_tensor_reduce(
                                attn[:, seg * L:(seg + 1) * L],
                                attnraw[:, seg * L:(seg + 1) * L],
                                mk_t[:, it * L:(it + 1) * L],
                                1.0, 0.0,
                                mybir.AluOpType.mult, mybir.AluOpType.add,
                                accum_out=sums[:, seg:seg + 1])
                        elif HOSTNORM:
                            nc.scalar.activation(
                                attn[:, seg * L:(seg + 1) * L], pv,
                                mybir.ActivationFunctionType.Exp)
                            nc.gpsimd.dma_start(
                                out_d[:, seg * L:(seg + 1) * L],
                                attn[:, seg * L:(seg + 1) * L])
                        else:
                            nc.scalar.activation(
                                attn[:, seg * L:(seg + 1) * L], pv,
                                mybir.ActivationFunctionType.Exp,
                                accum_out=sums[:, seg:seg + 1])
                    if EXPPAIR and stages == "full":
                        nc.scalar.activation(
                            attn[:, l * 2 * L:(l + 1) * 2 * L], ps2[:],
                            mybir.ActivationFunctionType.Exp)
                if stages == "dma":
                    nc.gpsimd.dma_start(out_d[:, :L],
                                        pk_t[:, :2 * L].bitcast(HT))
                    continue
                if stages == "mm":
                    nc.vector.tensor_scalar_max(attn[:, :L], pv, 0.0)
                    nc.gpsimd.dma_start(out_d[:, :L], attn[:, :L])
                    continue
                if EXPPAIR:
                    nc.vector.reduce_sum(
                        sums[:],
                        attn[:].rearrange("p (s j) -> p s j", j=L),
                        axis=mybir.AxisListType.X)
                if HOSTNORM:
                    continue     # out-DMAs already issued per segment
                nc.vector.reciprocal(recip[:], sums[:])
                out_eng = {"sp": nc.sync, "act": nc.scalar,
                           "pool": nc.gpsimd, "pool8": nc.gpsimd}[OUTQ]
                for seg in range(BH * 2):
                    nc.vector.tensor_scalar_mul(
                        attn[:, seg * L:(seg + 1) * L],
                        attn[:, seg * L:(seg + 1) * L],
                        recip[:, seg:seg + 1])
                    if OUTQ == "pool8":
                        out_eng.dma_start(out_d[:, seg * L:(seg + 1) * L],
                                          attn[:, seg * L:(seg + 1) * L])
                if OUTQ != "pool8":
                    out_eng.dma_start(out_d[:], attn[:])

    nc.compile()
    return nc


def _prep_core_inputs(q, k, mask, attention):
    """Host-side layout prep: per-core input dicts."""
    import ml_dtypes
    F8 = ml_dtypes.float8_e4m3
    f_tab, g_tab = _feature_tables()
    q = np.asarray(q, np.float32)
    k = np.asarray(k, np.float32)
    a = np.asarray(attention, np.float32).reshape(H, D)
    mask = np.asarray(mask).reshape(B, L, L)

    # features for all (b, h) at once: (B, H, L, D, R)
    qf = _interp_features(q, f_tab)
    kf = _interp_features(k, g_tab) * a[None, :, None, :, None]

    # (B, H, L, D, R) -> (B, H, C=R/2, 128, L) -> (B, H, 128, C, L)
    def chunked(X):
        X = X.transpose(0, 1, 4, 3, 2).reshape(B, H, R // 2, 128, L)
        return np.ascontiguousarray(X.transpose(0, 1, 3, 2, 4))

    qc = chunked(qf)    # (B, H, 128, C, L)
    kc = chunked(kf)

    id128 = np.eye(128, dtype=np.float16)
    in_maps = []
    for core in range(NCORES):
        pk = np.empty((128, BH * PLB), np.uint8)
        f8_sh = (BH, 128, L, C8) if DRSW else (BH, 128, C8, L)
        f8 = np.empty(f8_sh, F8)
        g8 = np.empty(f8_sh, F8)
        for l in range(BH):
            f = BH * core + l
            b, h = f // H, f % H
            base = l * PLB
            pk[:, base:base + 1024] = (
                qc[b, h, :, :C16].reshape(128, C16 * L)
                .astype(np.float16).view(np.uint8))
            pk[:, base + 1024:base + 2048] = (
                kc[b, h, :, :C16].reshape(128, C16 * L)
                .astype(np.float16).view(np.uint8))
            if DRSW:
                f8[l] = qc[b, h, :, C16:].transpose(0, 2, 1)
                g8[l] = kc[b, h, :, C16:].transpose(0, 2, 1)
            else:
                f8[l] = qc[b, h, :, C16:]
                g8[l] = kc[b, h, :, C16:]
            if F8PACK:
                # f8 region: [it][c][m] for the per-i-tile lhsT views
                f8p = (qc[b, h, :, C16:]                # (128, C8, 256)
                       .reshape(128, C8, 2, 128)        # (p, c, it, m)
                       .transpose(0, 2, 1, 3)           # (p, it, c, m)
                       .reshape(128, 512).astype(F8))
                pk[:, base + 2048:base + 2560] = f8p.view(np.uint8)
                # g8 region: [c][n]
                pk[:, base + 2560:base + 3072] = (
                    kc[b, h, :, C16:].reshape(128, 512)
                    .astype(F8).view(np.uint8))
        b = BH * core // H
        if MASKDVE:
            mb = np.where(mask[b], np.float32(0), np.float32(1)
                          ).astype(np.float16)
        else:
            mb = np.where(mask[b], MASK_NEG, np.float32(0)).astype(np.float16)
        mk = np.ascontiguousarray(
            np.concatenate([mb[:128], mb[128:]], axis=1))
        im = {"pk": pk, "mk": mk, "id128": id128}
        if not F8PACK:
            im["f8"], im["g8"] = f8, g8
        in_maps.append(im)
    return in_maps


def _get_runner():
    """Persistent jitted shard_map runner over 8 cores."""
    if "runner" in _cache:
        return _cache["runner"]

    import jax
    import concourse.mybir as mybir
    from jax.sharding import Mesh, PartitionSpec
    from jax.experimental.shard_map import shard_map
    from concourse import bass2jax

    bass2jax.install_neuronx_cc_hook()
    nc = _build_program()

    part_name = (nc.partition_id_tensor.name
                 if nc.partition_id_tensor else None)
    in_names, out_names, out_avals, zero_outs = [], [], [], []
    for alloc in nc.m.functions[0].allocations:
        if not isinstance(alloc, mybir.MemoryLocationSet):
            continue
        name = alloc.memorylocations[0].name
        if alloc.kind == "ExternalInput":
            if name != part_name:
                in_names.append(name)
        elif alloc.kind == "ExternalOutput":
            shape = tuple(alloc.tensor_shape)
            dtype = mybir.dt.np(alloc.dtype)
            out_names.append(name)
            out_avals.append(jax.core.ShapedArray(shape, dtype))
            zero_outs.append(np.zeros(shape, dtype))
    n_params = len(in_names)
    all_names = in_names + out_names
    if part_name is not None:
        all_names = all_names + [part_name]

    def _body(*args):
        operands = list(args)
        if part_name is not None:
            operands.append(bass2jax.partition_id_tensor())
        return tuple(bass2jax._bass_exec_p.bind(
            *operands,
            out_avals=tuple(out_avals),
            in_names=tuple(all_names),
            out_names=tuple(out_names),
            lowering_input_output_aliases=(),
            sim_require_finite=True,
            sim_require_nnan=True,
            nc=nc,
        ))

    devices = jax.devices()[:NCORES]
    mesh = Mesh(np.asarray(devices), ("core",))
    n_outs = len(out_names)
    sharded = jax.jit(
        shard_map(_body, mesh=mesh,
                  in_specs=(PartitionSpec("core"),) * (n_params + n_outs),
                  out_specs=(PartitionSpec("core"),) * n_outs,
                  check_rep=False),
        donate_argnums=tuple(range(n_params, n_params + n_outs)),
        keep_unused=True)

    def run(in_maps):
        concat_in = [
            np.concatenate([in_maps[c][nm] for c in range(NCORES)], axis=0)
            for nm in in_names]
        concat_zeros = [np.zeros((NCORES * z.shape[0], *z.shape[1:]), z.dtype)
                        for z in zero_outs]
        outs = sharded(*concat_in, *concat_zeros)
        return [
            {nm: np.asarray(outs[i]).reshape(NCORES, *out_avals[i].shape)[c]
             for i, nm in enumerate(out_names)}
            for c in range(NCORES)]

    run.sharded = sharded
    run.in_names = in_names
    run.zero_outs = zero_outs
    _cache["runner"] = run
    return run


def kernel(q, k, scale, mask, attention):
    results = _get_runner()(_prep_core_inputs(q, k, mask, attention))
    attn = np.empty((B, H, L, L), np.float32)
    for core in range(NCORES):
        o = results[core]["out"].astype(np.float32)   # (128, BH*2*L)
        o = o.reshape(128, BH, 2, L)
        if HOSTNORM:
            o = o / o.sum(axis=-1, keepdims=True)
        for l in range(BH):
            f = BH * core + l
            b, h = f // H, f % H
            attn[b, h, :128] = o[:, l, 0]
            attn[b, h, 128:] = o[:, l, 1]
    return attn
